# revision 29
# baseline (speedup 1.0000x reference)
"""Trainium2 Bass kernel for DynamicDeepSNN (3-layer LIF SNN, T=100).

Sharding: data-parallel over batch (1024 -> 128 per core, 8 cores).
Weights replicated. Time loop strictly sequential on-device.

Math per step (snntorch Leaky, reset-by-subtract, forward only):
    cur1 = x_t @ W1.T + b1 ;  m1 = 0.8*m1 + cur1 - s1_prev ;  s1 = (m1 > 1)
    cur2 = s1 @ W2.T + b2  ;  m2 = 0.8*m2 + cur2 - s2_prev ;  s2 = (m2 > 1)
    cur3 = s2 @ W3.T + b3  ;  m3 = 0.8*m3 + cur3 - s3_prev ;  s3 = (m3 > 1)

Device design (per core):
  Phase A (interleaved with B, 2 groups of lead): CUR1T[feat, t, batch] =
    W1 @ x_t.T (+b1), fp32 matmuls with N=512 moving dim, staged in SBUF.
  Phase B: staggered ticks (layer1 at step tau, layer2 at tau-1, layer3 at
    tau-2) so every tick's matmuls only consume spikes from earlier ticks.
    PSUM tile PB spans two banks: bank0 holds cur1 - s1_prev (ACT copies
    cur1 in, PE accumulates a bf16 -I @ s1; the bank's has_written bits are
    set once at tick 0 and never cleared), bank1 holds cur2 - s2_prev
    (first cur2 matmul uses start=True, clearing only bank1).  One fused
    scalar_tensor_tensor then updates m1|m2 = 0.8*m + psum across both
    banks, and one tensor op produces both spike planes.
    Layer3 lives transposed [NO=3 partitions, batch free]: its matmuls use
    3-column weight loads, with W3 split hi+lo into two bf16 matmuls
    (split error ~2^-18 — the same class as fp32 rounding; spikes are
    exactly representable in bf16).
  Biases: b1 folded into CUR1 during the PSUM->SBUF copy; b2/b3 via the
    exact-for-zero shift m~ = m - 5b (threshold 1 - 5b, init -5b), m3
    un-shifted on host.  When b2 == b3 == 0 (always true for this
    problem's setup_inputs) the spike compares use immediate 1.0 and the
    faster tensor_scalar path.
"""

import sys

for _p in ("/opt/trn_rl_repo",):
    if _p not in sys.path:
        sys.path.insert(0, _p)

import numpy as np
import ml_dtypes

import concourse.bass as bass
import concourse.mybir as mybir
from concourse.tile import TileContext
from concourse.bass_utils import run_bass_kernel_spmd

F32 = mybir.dt.float32
BF16 = mybir.dt.bfloat16
AF = mybir.ActivationFunctionType
ALU = mybir.AluOpType

T, B, NI, NH, NO = 100, 1024, 512, 256, 3
NCORES = 8
BC = B // NCORES  # 128 batch rows per core
BETA = 0.8
THR = 1.0
TG = 4  # timesteps per phase-A matmul group (N = TG*BC = 512)

NKC1 = NI // 128   # 4 K-chunks for layer 1
NMC = NH // 128    # 2 M-chunks for layers 1/2


# --------------------------------------------------------------------------
# Walrus in this toolchain accepts at most ONE embedded sync-wait (and one
# sync-update) per instruction (NEURON_ISA_TPB_EVENTS has a single slot);
# Tile emits instructions with several (e.g. the kernel-tail Drain).
# Legalize the BIR JSON: move extra waits onto single-wait NoOp carriers
# inserted just before (and extra updates onto NoOps just after).
# --------------------------------------------------------------------------
def _legalize_waits(bir: dict) -> dict:
    for fn in bir.get("functions", []):
        for blk in fn.get("blocks", []):
            out = []
            for ins in blk.get("instructions", []):
                si = ins.get("sync_info")
                ws = (si or {}).get("on_wait") or []
                if len(ws) > 1:
                    for k, w in enumerate(ws[:-1]):
                        out.append({
                            "debug": ins.get("debug", 0),
                            "engine": ins["engine"],
                            "ins": [],
                            "outs": [],
                            "name": f"{ins['name']}-lw{k}",
                            "opcode": "NoOp",
                            "sync_info": {"on_update": [], "on_wait": [w]},
                        })
                    si["on_wait"] = [ws[-1]]
                out.append(ins)
                us = (si or {}).get("on_update") or []
                if len(us) > 1:
                    si["on_update"] = [us[0]]
                    for k, u in enumerate(us[1:]):
                        out.append({
                            "debug": ins.get("debug", 0),
                            "engine": ins["engine"],
                            "ins": [],
                            "outs": [],
                            "name": f"{ins['name']}-lu{k}",
                            "opcode": "NoOp",
                            "sync_info": {"on_update": [u], "on_wait": []},
                        })
            blk["instructions"] = out
    return bir


def _install_legalizer():
    import orjson
    from concourse import bass2jax as _b2j

    target = _b2j.compile_bir_kernel
    if getattr(target, "_wait_legalizer", False):
        return
    def patched(bir_json, *a, **kw):
        d = orjson.loads(bir_json)
        _legalize_waits(d)
        return target(orjson.dumps(d), *a, **kw)
    patched._wait_legalizer = True
    _b2j.compile_bir_kernel = patched


_install_legalizer()


def build_bass(t_steps=T, fast_thr=True):
    t = t_steps
    ngroups = t // TG
    assert t % TG == 0
    nc = bass.Bass()

    xt_d = nc.dram_tensor("xt", [NI, t, BC], F32, kind="ExternalInput")
    w1t_d = nc.dram_tensor("w1t", [NI, NH], F32, kind="ExternalInput")
    w2t_d = nc.dram_tensor("w2t", [NH, NH], F32, kind="ExternalInput")
    w3hi_d = nc.dram_tensor("w3hi", [NH, NO], BF16, kind="ExternalInput")
    w3lo_d = nc.dram_tensor("w3lo", [NH, NO], BF16, kind="ExternalInput")
    b1s_d = nc.dram_tensor("b1s", [128, NMC], F32, kind="ExternalInput")
    minit_d = nc.dram_tensor("minit", [128, 2 * NH], F32, kind="ExternalInput")
    m3init_d = nc.dram_tensor("m3init", [NO, BC], F32, kind="ExternalInput")
    thr2_d = nc.dram_tensor("thr2", [128, NH], F32, kind="ExternalInput")
    thr3_d = nc.dram_tensor("thr3", [NO, 1], F32, kind="ExternalInput")
    negi_d = nc.dram_tensor("negi", [128, 128], BF16, kind="ExternalInput")
    spk_d = nc.dram_tensor("spk", [NO, t * BC], F32, kind="ExternalOutput")
    m3_d = nc.dram_tensor("m3", [NO, BC], F32, kind="ExternalOutput")

    with TileContext(nc) as tc:
        with (
            tc.tile_pool(name="const", bufs=1) as const,
            tc.tile_pool(name="xstream", bufs=4) as xstream,
            tc.tile_pool(name="psA", bufs=2, space="PSUM") as psA,
            tc.tile_pool(name="psB", bufs=1, space="PSUM") as psB,
        ):
            # ---- persistent tiles ----
            W1Ts = const.tile([128, NKC1, NH], F32)
            W2Ts = const.tile([128, NMC, NH], F32)
            W3Hi = const.tile([128, NMC, NO], BF16)
            W3Lo = const.tile([128, NMC, NO], BF16)
            B1s = const.tile([128, NMC], F32)
            NegI = const.tile([128, 128], BF16)
            M = const.tile([128, 2 * NH], F32)       # m1 | m2  [feat, batch]
            S1 = const.tile([128, NH], F32)          # s1 (fp32, feeds W2 matmul)
            S16_1 = const.tile([128, NH], BF16)      # s1 in bf16 (reset matmul)
            S16 = const.tile([128, NH], BF16)        # s2 in bf16 (reset + W3)
            M3 = const.tile([NO, BC], F32)
            S3 = const.tile([NO, BC], F32)
            THR2 = const.tile([128, NH], F32)
            THR3 = const.tile([NO, 1], F32)
            CUR1 = const.tile([128, NMC, t, BC], F32)
            OUT = const.tile([NO, t * BC], F32)

            for _kc in range(NKC1):
                nc.sync.dma_start(
                    out=W1Ts[:, _kc],
                    in_=w1t_d.rearrange("(kc p) m -> p kc m", p=128)[:, _kc],
                )

            # PB: [128, 2 banks, 256]: bank0 = cur1 - s1_prev (persistent
            # has_written bits), bank1 = cur2 - s2_prev (cleared per tick).
            nc.sync.dma_start(out=W2Ts[:], in_=w2t_d.rearrange("(kc p) m -> p kc m", p=128))
            nc.sync.dma_start(out=W3Hi[:], in_=w3hi_d.rearrange("(kc p) m -> p kc m", p=128))
            nc.sync.dma_start(out=W3Lo[:], in_=w3lo_d.rearrange("(kc p) m -> p kc m", p=128))
            nc.sync.dma_start(out=B1s[:], in_=b1s_d[:])
            nc.sync.dma_start(out=NegI[:], in_=negi_d[:])
            nc.sync.dma_start(out=M[:], in_=minit_d[:])
            nc.sync.dma_start(out=M3[:], in_=m3init_d[:])
            nc.sync.dma_start(out=THR2[:], in_=thr2_d[:])
            nc.sync.dma_start(out=THR3[:], in_=thr3_d[:])
            nc.vector.memset(S1[:], 0.0)
            nc.vector.memset(S16_1[:], 0.0)
            nc.vector.memset(S16[:], 0.0)
            nc.vector.memset(S3[:], 0.0)

            PB = psB.tile([128, 2, 512], F32, tag="PB")
            P3 = psB.tile([NO, BC], F32, tag="P3")

            xg_tiles = {}

            def emit_group_dma(g):
                if g in xg_tiles:
                    return
                xg = xstream.tile([128, NKC1, TG, BC], F32, tag="xg")
                xg_tiles[g] = xg
                xsrc = xt_d.rearrange("(kc p) t b -> p kc t b", p=128)[
                    :, :, g * TG:(g + 1) * TG, :
                ]
                # split per K-chunk: 4 queues move the group in parallel
                # and the first matmul only waits for the first quarter
                for kc in range(NKC1):
                    nc.sync.dma_start(out=xg[:, kc], in_=xsrc[:, kc])

            def emit_group_mms(g):
                xg = xg_tiles.pop(g)
                for mc in range(NMC):
                    pa = psA.tile([128, TG * BC], F32, tag=f"pa{mc}")
                    for kc in range(NKC1):
                        nc.tensor.matmul(
                            pa[:],
                            lhsT=W1Ts[:, kc, mc * 128:(mc + 1) * 128],
                            rhs=xg[:, kc],
                            start=(kc == 0),
                            stop=(kc == NKC1 - 1),
                        )
                    nc.scalar.activation(
                        out=CUR1[:, mc, g * TG:(g + 1) * TG, :],
                        in_=pa[:],
                        func=AF.Identity,
                        bias=B1s[:, mc:mc + 1],
                        scale=1.0,
                    )

            def emit_tick(tau):
                l1 = tau < t
                l2 = 1 <= tau <= t
                l3 = 2 <= tau <= t + 1

                # ---- feed PSUM ----
                half = (tau % 2) * NH
                if tau == 0:
                    # set has_written bits on both halves of bank0 once
                    # (S16_1 is still zero, so this writes zeros); ACT then
                    # overwrites with cur1 leaving the bits set.
                    nc.tensor.matmul(
                        PB[:, 0, 0:NH], lhsT=NegI[:], rhs=S16_1[:],
                        start=True, stop=False, skip_group_check=True,
                    )
                    nc.tensor.matmul(
                        PB[:, 0, NH:2 * NH], lhsT=NegI[:], rhs=S16_1[:],
                        start=False, stop=True, skip_group_check=True,
                    )
                    nc.scalar.copy(
                        out=PB[:, 0, 0:2 * NH],
                        in_=CUR1[:, :, 0:2, :].rearrange("p m t b -> p t m b"),
                    )
                elif l1:
                    if tau % 2 == 0:
                        # one ACT copy delivers cur1 for this tick + the next
                        ncopy = 2 if tau + 1 < t else 1
                        nc.scalar.copy(
                            out=PB[:, 0, 0:ncopy * NH],
                            in_=CUR1[:, :, tau:tau + ncopy, :].rearrange(
                                "p m t b -> p t m b"),
                        )
                    nc.tensor.matmul(
                        PB[:, 0, half:half + NH], lhsT=NegI[:], rhs=S16_1[:],
                        start=False, stop=True, skip_group_check=True,
                    )
                if l2:
                    first = True
                    for mc in range(NMC):
                        for kc in range(NMC):
                            nc.tensor.matmul(
                                PB[:, 1, mc * 128:(mc + 1) * 128],
                                lhsT=W2Ts[:, kc, mc * 128:(mc + 1) * 128],
                                rhs=S1[:, kc * 128:(kc + 1) * 128],
                                start=first,
                                stop=False,
                                skip_group_check=True,
                            )
                            first = False
                    # reset with s2(tau-2): S16 still holds the previous
                    # tick's spike plane (this tick's write comes later)
                    nc.tensor.matmul(
                        PB[:, 1, 0:NH], lhsT=NegI[:], rhs=S16[:],
                        start=False, stop=True, skip_group_check=True,
                    )
                if l3:
                    for kc in range(NMC):
                        nc.tensor.matmul(
                            P3[:],
                            lhsT=W3Hi[:, kc, :],
                            rhs=S16[:, kc * 128:(kc + 1) * 128],
                            start=(kc == 0),
                            stop=False,
                            skip_group_check=True,
                        )
                        nc.tensor.matmul(
                            P3[:],
                            lhsT=W3Lo[:, kc, :],
                            rhs=S16[:, kc * 128:(kc + 1) * 128],
                            start=False,
                            stop=(kc == NMC - 1),
                            skip_group_check=True,
                        )


                # ---- DVE: fused membrane update + spikes ----
                if l1 or l2:
                    lo = 0 if l1 else NH
                    hi = 2 * NH if l2 else NH
                    PBf = PB.rearrange("p a b -> p (a b)")
                    if l1 and l2:
                        in1 = (PB[:, 0:2, 0:NH] if half == 0
                               else PBf[:, NH:3 * NH])
                    elif l1:
                        in1 = PBf[:, half:half + NH]
                    else:
                        in1 = PBf[:, 2 * NH:3 * NH]
                    nc.vector.scalar_tensor_tensor(
                        out=M[:, lo:hi], in0=M[:, lo:hi], scalar=BETA,
                        in1=in1, op0=ALU.mult, op1=ALU.add,
                    )
                if l1:
                    nc.vector.tensor_scalar(
                        out=S1[:], in0=M[:, 0:NH],
                        scalar1=THR, scalar2=None, op0=ALU.is_gt,
                    )
                    nc.scalar.copy(out=S16_1[:], in_=S1[:])
                if l2:
                    # s2 only ever feeds bf16 matmuls (reset + W3): write
                    # the bf16 spike plane directly
                    if fast_thr:
                        nc.vector.tensor_scalar(
                            out=S16[:], in0=M[:, NH:2 * NH],
                            scalar1=THR, scalar2=None, op0=ALU.is_gt,
                        )
                    else:
                        nc.vector.tensor_tensor(
                            out=S16[:], in0=M[:, NH:2 * NH],
                            in1=THR2[:], op=ALU.is_gt,
                        )
                if l3:
                    # u3 = 0.8*m3 - s3_prev (S3 still holds last tick's
                    # spike plane), then m3 = u3 + cur3
                    nc.vector.scalar_tensor_tensor(
                        out=M3[:], in0=M3[:], scalar=BETA,
                        in1=S3[:], op0=ALU.mult, op1=ALU.subtract,
                    )
                    nc.vector.tensor_tensor(
                        out=M3[:], in0=M3[:], in1=P3[:], op=ALU.add,
                    )
                    nc.vector.tensor_scalar(
                        out=S3[:], in0=M3[:],
                        scalar1=(THR if fast_thr else THR3[:]),
                        scalar2=None, op0=ALU.is_gt,
                    )
                # ---- ACT: output staging ----
                if l3:
                    nc.scalar.copy(
                        out=OUT[:, (tau - 2) * BC:(tau - 1) * BC], in_=S3[:],
                    )

            # interleave: DMA runs DLEAD groups ahead, matmuls MLEAD ahead
            DLEAD, MLEAD = min(4, ngroups), 1
            for g in range(DLEAD):
                emit_group_dma(g)
            for g in range(ngroups):
                if g + DLEAD < ngroups:
                    emit_group_dma(g + DLEAD)
                emit_group_mms(g)
                if g >= MLEAD:
                    for tau in range(TG * (g - MLEAD), TG * (g - MLEAD + 1)):
                        emit_tick(tau)
            for tau in range(TG * (ngroups - MLEAD), t + 2):
                emit_tick(tau)

            nc.sync.dma_start(out=spk_d[:], in_=OUT[:])
            nc.sync.dma_start(out=m3_d[:], in_=M3[:])

    return nc


_NC_CACHE = {}


def _get_nc(t_steps, fast_thr):
    key = (t_steps, fast_thr)
    if key not in _NC_CACHE:
        _NC_CACHE[key] = build_bass(t_steps, fast_thr)
    return _NC_CACHE[key]


def _prep_inputs(x, W1, b1, W2, b2, W3, b3):
    w1t = np.ascontiguousarray(W1.T)  # (NI, NH)
    w2t = np.ascontiguousarray(W2.T)  # (NH, NH)
    w3t = np.ascontiguousarray(W3.T).astype(np.float32)  # (NH, NO)
    w3hi = w3t.astype(ml_dtypes.bfloat16)
    w3lo = (w3t - w3hi.astype(np.float32)).astype(ml_dtypes.bfloat16)
    b1s = np.ascontiguousarray(b1.reshape(NMC, 128).T)  # (128, NMC)
    # shift trick for b2/b3: m~ = m - 5*b, threshold 1 - 5*b, init -5*b
    shift2 = 5.0 * b2  # (NH,)
    shift3 = 5.0 * b3  # (NO,)
    m2init = np.broadcast_to((-shift2.reshape(NMC, 128).T)[:, :, None],
                             (128, NMC, 128))
    minit = np.zeros((128, 2 * NH), np.float32)
    minit[:, NH:] = m2init.reshape(128, NH)
    m3init = np.ascontiguousarray(
        np.broadcast_to(-shift3[:, None], (NO, BC)), dtype=np.float32)
    t2 = np.broadcast_to((THR - shift2.reshape(NMC, 128).T)[:, :, None],
                         (128, NMC, 128))
    thr2 = np.ascontiguousarray(t2.reshape(128, NH), dtype=np.float32)
    thr3 = np.ascontiguousarray((THR - shift3)[:, None], dtype=np.float32)
    negi = (-np.eye(128, dtype=np.float32)).astype(ml_dtypes.bfloat16)

    shared = dict(w1t=w1t, w2t=w2t, w3hi=w3hi, w3lo=w3lo, b1s=b1s,
                  minit=minit, m3init=m3init, thr2=thr2, thr3=thr3,
                  negi=negi)
    in_maps = []
    for c in range(NCORES):
        xc = x[:, c * BC:(c + 1) * BC, :]                 # (t, BC, NI)
        xt = np.ascontiguousarray(xc.transpose(2, 0, 1))  # (NI, t, BC)
        m = dict(shared)
        m["xt"] = xt
        in_maps.append(m)
    return in_maps


def _run(x, W1, b1, W2, b2, W3, b3, **spmd_kwargs):
    x = np.asarray(x, np.float32)
    W1 = np.asarray(W1, np.float32); b1 = np.asarray(b1, np.float32)
    W2 = np.asarray(W2, np.float32); b2 = np.asarray(b2, np.float32)
    W3 = np.asarray(W3, np.float32); b3 = np.asarray(b3, np.float32)
    t = x.shape[0]
    fast_thr = not (b2.any() or b3.any())
    nc = _get_nc(t, fast_thr)
    in_maps = _prep_inputs(x, W1, b1, W2, b2, W3, b3)
    bkr = run_bass_kernel_spmd(nc, in_maps, list(range(NCORES)), **spmd_kwargs)
    res = bkr.results
    spk = np.empty((t, B, NO), np.float32)
    m3 = np.empty((B, NO), np.float32)
    for c in range(NCORES):
        spk[:, c * BC:(c + 1) * BC, :] = (
            res[c]["spk"].reshape(NO, t, BC).transpose(1, 2, 0)
        )
        m3[c * BC:(c + 1) * BC, :] = res[c]["m3"].T + 5.0 * b3[None, :]
    return (spk, m3), bkr


def kernel(x, W1, b1, W2, b2, W3, b3):
    out, _ = _run(x, W1, b1, W2, b2, W3, b3)
    return out


# revision 31
# speedup vs baseline: 1.0526x; 1.0526x over previous
"""Trainium2 Bass kernel for DynamicDeepSNN (3-layer LIF SNN, T=100).

Sharding: data-parallel over batch (1024 -> 128 per core, 8 cores).
Weights replicated. Time loop strictly sequential on-device.

Math per step (snntorch Leaky, reset-by-subtract, forward only):
    cur1 = x_t @ W1.T + b1 ;  m1 = 0.8*m1 + cur1 - s1_prev ;  s1 = (m1 > 1)
    cur2 = s1 @ W2.T + b2  ;  m2 = 0.8*m2 + cur2 - s2_prev ;  s2 = (m2 > 1)
    cur3 = s2 @ W3.T + b3  ;  m3 = 0.8*m3 + cur3 - s3_prev ;  s3 = (m3 > 1)

Device design (per core):
  Phase A (interleaved with B, 2 groups of lead): CUR1T[feat, t, batch] =
    W1 @ x_t.T (+b1), fp32 matmuls with N=512 moving dim, staged in SBUF.
  Phase B: staggered ticks (layer1 at step tau, layer2 at tau-1, layer3 at
    tau-2) so every tick's matmuls only consume spikes from earlier ticks.
    PSUM tile PB spans two banks: bank0 holds cur1 - s1_prev (ACT copies
    cur1 in, PE accumulates a bf16 -I @ s1; the bank's has_written bits are
    set once at tick 0 and never cleared), bank1 holds cur2 - s2_prev
    (first cur2 matmul uses start=True, clearing only bank1).  One fused
    scalar_tensor_tensor then updates m1|m2 = 0.8*m + psum across both
    banks, and one tensor op produces both spike planes.
    Layer3 lives transposed [NO=3 partitions, batch free]: its matmuls use
    3-column weight loads, with W3 split hi+lo into two bf16 matmuls
    (split error ~2^-18 — the same class as fp32 rounding; spikes are
    exactly representable in bf16).
  Biases: b1 folded into CUR1 during the PSUM->SBUF copy; b2/b3 via the
    exact-for-zero shift m~ = m - 5b (threshold 1 - 5b, init -5b), m3
    un-shifted on host.  When b2 == b3 == 0 (always true for this
    problem's setup_inputs) the spike compares use immediate 1.0 and the
    faster tensor_scalar path.
"""

import sys

for _p in ("/opt/trn_rl_repo",):
    if _p not in sys.path:
        sys.path.insert(0, _p)

import numpy as np
import ml_dtypes

import concourse.bass as bass
import concourse.mybir as mybir
from concourse.tile import TileContext
from concourse.bass_utils import run_bass_kernel_spmd

F32 = mybir.dt.float32
BF16 = mybir.dt.bfloat16
AF = mybir.ActivationFunctionType
ALU = mybir.AluOpType

T, B, NI, NH, NO = 100, 1024, 512, 256, 3
NCORES = 8
BC = B // NCORES  # 128 batch rows per core
BETA = 0.8
THR = 1.0
TG = 4  # timesteps per phase-A matmul group (N = TG*BC = 512)

NKC1 = NI // 128   # 4 K-chunks for layer 1
NMC = NH // 128    # 2 M-chunks for layers 1/2


# --------------------------------------------------------------------------
# Walrus in this toolchain accepts at most ONE embedded sync-wait (and one
# sync-update) per instruction (NEURON_ISA_TPB_EVENTS has a single slot);
# Tile emits instructions with several (e.g. the kernel-tail Drain).
# Legalize the BIR JSON: move extra waits onto single-wait NoOp carriers
# inserted just before (and extra updates onto NoOps just after).
# --------------------------------------------------------------------------
def _legalize_waits(bir: dict) -> dict:
    for fn in bir.get("functions", []):
        for blk in fn.get("blocks", []):
            out = []
            for ins in blk.get("instructions", []):
                si = ins.get("sync_info")
                ws = (si or {}).get("on_wait") or []
                if len(ws) > 1:
                    for k, w in enumerate(ws[:-1]):
                        out.append({
                            "debug": ins.get("debug", 0),
                            "engine": ins["engine"],
                            "ins": [],
                            "outs": [],
                            "name": f"{ins['name']}-lw{k}",
                            "opcode": "NoOp",
                            "sync_info": {"on_update": [], "on_wait": [w]},
                        })
                    si["on_wait"] = [ws[-1]]
                out.append(ins)
                us = (si or {}).get("on_update") or []
                if len(us) > 1:
                    si["on_update"] = [us[0]]
                    for k, u in enumerate(us[1:]):
                        out.append({
                            "debug": ins.get("debug", 0),
                            "engine": ins["engine"],
                            "ins": [],
                            "outs": [],
                            "name": f"{ins['name']}-lu{k}",
                            "opcode": "NoOp",
                            "sync_info": {"on_update": [u], "on_wait": []},
                        })
            blk["instructions"] = out
    return bir


def _install_legalizer():
    import orjson
    from concourse import bass2jax as _b2j

    target = _b2j.compile_bir_kernel
    if getattr(target, "_wait_legalizer", False):
        return
    def patched(bir_json, *a, **kw):
        d = orjson.loads(bir_json)
        _legalize_waits(d)
        return target(orjson.dumps(d), *a, **kw)
    patched._wait_legalizer = True
    _b2j.compile_bir_kernel = patched


_install_legalizer()


def build_bass(t_steps=T, fast_thr=True):
    t = t_steps
    ngroups = t // TG
    assert t % TG == 0
    nc = bass.Bass()

    xt_d = nc.dram_tensor("xt", [NI, t, BC], F32, kind="ExternalInput")
    w1t_d = nc.dram_tensor("w1t", [NI, NH], F32, kind="ExternalInput")
    w2t_d = nc.dram_tensor("w2t", [NH, NH], F32, kind="ExternalInput")
    w3hi_d = nc.dram_tensor("w3hi", [NH, NO], BF16, kind="ExternalInput")
    w3lo_d = nc.dram_tensor("w3lo", [NH, NO], BF16, kind="ExternalInput")
    b1s_d = nc.dram_tensor("b1s", [128, NMC], F32, kind="ExternalInput")
    minit_d = nc.dram_tensor("minit", [128, 2 * NH], F32, kind="ExternalInput")
    m3init_d = nc.dram_tensor("m3init", [NO, BC], F32, kind="ExternalInput")
    thr2_d = nc.dram_tensor("thr2", [128, NH], F32, kind="ExternalInput")
    thr3_d = nc.dram_tensor("thr3", [NO, 1], F32, kind="ExternalInput")
    negi_d = nc.dram_tensor("negi", [128, 128], BF16, kind="ExternalInput")
    spk_d = nc.dram_tensor("spk", [NO, t * BC], F32, kind="ExternalOutput")
    m3_d = nc.dram_tensor("m3", [NO, BC], F32, kind="ExternalOutput")

    with TileContext(nc) as tc:
        with (
            tc.tile_pool(name="const", bufs=1) as const,
            tc.tile_pool(name="xstream", bufs=4) as xstream,
            tc.tile_pool(name="psA", bufs=2, space="PSUM") as psA,
            tc.tile_pool(name="psB", bufs=1, space="PSUM") as psB,
        ):
            # ---- persistent tiles ----
            W1Ts = const.tile([128, NKC1, NH], F32)
            W2Ts = const.tile([128, NMC, NH], F32)
            W3Hi = const.tile([128, NMC, NO], BF16)
            W3Lo = const.tile([128, NMC, NO], BF16)
            B1s = const.tile([128, NMC], F32)
            NegI = const.tile([128, 128], BF16)
            M = const.tile([128, 2 * NH], F32)       # m1 | m2  [feat, batch]
            S1 = const.tile([128, NH], F32)          # s1 (fp32, feeds W2 matmul)
            S16_1 = const.tile([128, NH], BF16)      # s1 in bf16 (reset matmul)
            S16 = const.tile([128, NH], BF16)        # s2 in bf16 (reset + W3)
            M3 = const.tile([NO, BC], F32)
            S3 = const.tile([NO, BC], F32)
            THR2 = const.tile([128, NH], F32)
            THR3 = const.tile([NO, 1], F32)
            CUR1 = const.tile([128, NMC, t, BC], F32)
            OUT = const.tile([NO, t * BC], F32)

            for _kc in range(NKC1):
                nc.sync.dma_start(
                    out=W1Ts[:, _kc],
                    in_=w1t_d.rearrange("(kc p) m -> p kc m", p=128)[:, _kc],
                )

            # PB: [128, 2 banks, 256]: bank0 = cur1 - s1_prev (persistent
            # has_written bits), bank1 = cur2 - s2_prev (cleared per tick).

            PB = psB.tile([128, 2, 512], F32, tag="PB")
            P3 = psB.tile([NO, BC], F32, tag="P3")

            xg_tiles = {}

            def emit_group_dma(g):
                if g in xg_tiles:
                    return
                xg = xstream.tile([128, NKC1, TG, BC], F32, tag="xg")
                xg_tiles[g] = xg
                xsrc = xt_d.rearrange("(kc p) t b -> p kc t b", p=128)[
                    :, :, g * TG:(g + 1) * TG, :
                ]
                # split per K-chunk: 4 queues move the group in parallel
                # and the first matmul only waits for the first quarter
                for kc in range(NKC1):
                    nc.sync.dma_start(out=xg[:, kc], in_=xsrc[:, kc])

            def emit_group_mms(g):
                xg = xg_tiles.pop(g)
                for mc in range(NMC):
                    pa = psA.tile([128, TG * BC], F32, tag=f"pa{mc}")
                    for kc in range(NKC1):
                        nc.tensor.matmul(
                            pa[:],
                            lhsT=W1Ts[:, kc, mc * 128:(mc + 1) * 128],
                            rhs=xg[:, kc],
                            start=(kc == 0),
                            stop=(kc == NKC1 - 1),
                        )
                    nc.scalar.activation(
                        out=CUR1[:, mc, g * TG:(g + 1) * TG, :],
                        in_=pa[:],
                        func=AF.Identity,
                        bias=B1s[:, mc:mc + 1],
                        scale=1.0,
                    )

            def emit_tick(tau):
                l1 = tau < t
                l2 = 1 <= tau <= t
                l3 = 2 <= tau <= t + 1

                # ---- feed PSUM ----
                if tau == 0:
                    # set has_written bits on bank0 once (S16_1 is still
                    # zero here, so this writes zeros); ACT then overwrites
                    # with cur1(0) leaving the bits set.
                    nc.tensor.matmul(
                        PB[:, 0, 0:NH], lhsT=NegI[:], rhs=S16_1[:],
                        start=True, stop=True, skip_group_check=True,
                    )
                    nc.scalar.copy(out=PB[:, 0, 0:NH], in_=CUR1[:, :, 0, :])
                elif l1:
                    nc.scalar.copy(out=PB[:, 0, 0:NH], in_=CUR1[:, :, tau, :])
                    nc.tensor.matmul(
                        PB[:, 0, 0:NH], lhsT=NegI[:], rhs=S16_1[:],
                        start=False, stop=True, skip_group_check=True,
                    )
                if l2:
                    first = True
                    for mc in range(NMC):
                        for kc in range(NMC):
                            nc.tensor.matmul(
                                PB[:, 1, mc * 128:(mc + 1) * 128],
                                lhsT=W2Ts[:, kc, mc * 128:(mc + 1) * 128],
                                rhs=S1[:, kc * 128:(kc + 1) * 128],
                                start=first,
                                stop=False,
                                skip_group_check=True,
                            )
                            first = False
                    # reset with s2(tau-2): S16 still holds the previous
                    # tick's spike plane (this tick's write comes later)
                    nc.tensor.matmul(
                        PB[:, 1, 0:NH], lhsT=NegI[:], rhs=S16[:],
                        start=False, stop=True, skip_group_check=True,
                    )
                if l3:
                    for kc in range(NMC):
                        nc.tensor.matmul(
                            P3[:],
                            lhsT=W3Hi[:, kc, :],
                            rhs=S16[:, kc * 128:(kc + 1) * 128],
                            start=(kc == 0),
                            stop=False,
                            skip_group_check=True,
                        )
                        nc.tensor.matmul(
                            P3[:],
                            lhsT=W3Lo[:, kc, :],
                            rhs=S16[:, kc * 128:(kc + 1) * 128],
                            start=False,
                            stop=(kc == NMC - 1),
                            skip_group_check=True,
                        )


                # ---- DVE: fused membrane update + spikes ----
                if l1 or l2:
                    lo = 0 if l1 else NH
                    hi = 2 * NH if l2 else NH
                    nc.vector.scalar_tensor_tensor(
                        out=M[:, lo:hi], in0=M[:, lo:hi], scalar=BETA,
                        in1=PB[:, (0 if l1 else 1):(2 if l2 else 1), 0:NH],
                        op0=ALU.mult, op1=ALU.add,
                    )
                if l1:
                    nc.vector.tensor_scalar(
                        out=S1[:], in0=M[:, 0:NH],
                        scalar1=THR, scalar2=None, op0=ALU.is_gt,
                    )
                    nc.scalar.copy(out=S16_1[:], in_=S1[:])
                if l2:
                    # s2 only ever feeds bf16 matmuls (reset + W3): write
                    # the bf16 spike plane directly
                    if fast_thr:
                        nc.vector.tensor_scalar(
                            out=S16[:], in0=M[:, NH:2 * NH],
                            scalar1=THR, scalar2=None, op0=ALU.is_gt,
                        )
                    else:
                        nc.vector.tensor_tensor(
                            out=S16[:], in0=M[:, NH:2 * NH],
                            in1=THR2[:], op=ALU.is_gt,
                        )
                if l3:
                    # u3 = 0.8*m3 - s3_prev (S3 still holds last tick's
                    # spike plane), then m3 = u3 + cur3
                    nc.vector.scalar_tensor_tensor(
                        out=M3[:], in0=M3[:], scalar=BETA,
                        in1=S3[:], op0=ALU.mult, op1=ALU.subtract,
                    )
                    nc.vector.tensor_tensor(
                        out=M3[:], in0=M3[:], in1=P3[:], op=ALU.add,
                    )
                    nc.vector.tensor_scalar(
                        out=S3[:], in0=M3[:],
                        scalar1=(THR if fast_thr else THR3[:]),
                        scalar2=None, op0=ALU.is_gt,
                    )
                # ---- ACT: output staging ----
                if l3:
                    nc.scalar.copy(
                        out=OUT[:, (tau - 2) * BC:(tau - 1) * BC], in_=S3[:],
                    )
                    done = tau - 1  # steps [0, done) staged in OUT
                    if done % 32 == 0 or done == t:
                        lo_s = done - 32 if done % 32 == 0 else (done // 32) * 32
                        nc.sync.dma_start(
                            out=spk_d[:, lo_s * BC:done * BC],
                            in_=OUT[:, lo_s * BC:done * BC],
                        )

            # interleave: DMA runs DLEAD groups ahead, matmuls MLEAD ahead
            DLEAD, MLEAD = min(4, ngroups), 1
            for g in range(min(2, DLEAD)):
                emit_group_dma(g)
            nc.sync.dma_start(out=W2Ts[:], in_=w2t_d.rearrange("(kc p) m -> p kc m", p=128))
            for g in range(2, DLEAD):
                emit_group_dma(g)
            nc.sync.dma_start(out=W3Hi[:], in_=w3hi_d.rearrange("(kc p) m -> p kc m", p=128))
            nc.sync.dma_start(out=W3Lo[:], in_=w3lo_d.rearrange("(kc p) m -> p kc m", p=128))
            nc.sync.dma_start(out=B1s[:], in_=b1s_d[:])
            nc.sync.dma_start(out=NegI[:], in_=negi_d[:])
            nc.sync.dma_start(out=M[:], in_=minit_d[:])
            nc.sync.dma_start(out=M3[:], in_=m3init_d[:])
            nc.sync.dma_start(out=THR2[:], in_=thr2_d[:])
            nc.sync.dma_start(out=THR3[:], in_=thr3_d[:])
            nc.vector.memset(S1[:], 0.0)
            nc.vector.memset(S16_1[:], 0.0)
            nc.vector.memset(S16[:], 0.0)
            nc.vector.memset(S3[:], 0.0)
            for g in range(ngroups):
                if g + DLEAD < ngroups:
                    emit_group_dma(g + DLEAD)
                emit_group_mms(g)
                if g >= MLEAD:
                    for tau in range(TG * (g - MLEAD), TG * (g - MLEAD + 1)):
                        emit_tick(tau)
            for tau in range(TG * (ngroups - MLEAD), t + 2):
                emit_tick(tau)

            nc.sync.dma_start(out=m3_d[:], in_=M3[:])

    return nc


_NC_CACHE = {}


def _get_nc(t_steps, fast_thr):
    key = (t_steps, fast_thr)
    if key not in _NC_CACHE:
        _NC_CACHE[key] = build_bass(t_steps, fast_thr)
    return _NC_CACHE[key]


def _prep_inputs(x, W1, b1, W2, b2, W3, b3):
    w1t = np.ascontiguousarray(W1.T)  # (NI, NH)
    w2t = np.ascontiguousarray(W2.T)  # (NH, NH)
    w3t = np.ascontiguousarray(W3.T).astype(np.float32)  # (NH, NO)
    w3hi = w3t.astype(ml_dtypes.bfloat16)
    w3lo = (w3t - w3hi.astype(np.float32)).astype(ml_dtypes.bfloat16)
    b1s = np.ascontiguousarray(b1.reshape(NMC, 128).T)  # (128, NMC)
    # shift trick for b2/b3: m~ = m - 5*b, threshold 1 - 5*b, init -5*b
    shift2 = 5.0 * b2  # (NH,)
    shift3 = 5.0 * b3  # (NO,)
    m2init = np.broadcast_to((-shift2.reshape(NMC, 128).T)[:, :, None],
                             (128, NMC, 128))
    minit = np.zeros((128, 2 * NH), np.float32)
    minit[:, NH:] = m2init.reshape(128, NH)
    m3init = np.ascontiguousarray(
        np.broadcast_to(-shift3[:, None], (NO, BC)), dtype=np.float32)
    t2 = np.broadcast_to((THR - shift2.reshape(NMC, 128).T)[:, :, None],
                         (128, NMC, 128))
    thr2 = np.ascontiguousarray(t2.reshape(128, NH), dtype=np.float32)
    thr3 = np.ascontiguousarray((THR - shift3)[:, None], dtype=np.float32)
    negi = (-np.eye(128, dtype=np.float32)).astype(ml_dtypes.bfloat16)

    shared = dict(w1t=w1t, w2t=w2t, w3hi=w3hi, w3lo=w3lo, b1s=b1s,
                  minit=minit, m3init=m3init, thr2=thr2, thr3=thr3,
                  negi=negi)
    in_maps = []
    for c in range(NCORES):
        xc = x[:, c * BC:(c + 1) * BC, :]                 # (t, BC, NI)
        xt = np.ascontiguousarray(xc.transpose(2, 0, 1))  # (NI, t, BC)
        m = dict(shared)
        m["xt"] = xt
        in_maps.append(m)
    return in_maps


def _run(x, W1, b1, W2, b2, W3, b3, **spmd_kwargs):
    x = np.asarray(x, np.float32)
    W1 = np.asarray(W1, np.float32); b1 = np.asarray(b1, np.float32)
    W2 = np.asarray(W2, np.float32); b2 = np.asarray(b2, np.float32)
    W3 = np.asarray(W3, np.float32); b3 = np.asarray(b3, np.float32)
    t = x.shape[0]
    fast_thr = not (b2.any() or b3.any())
    nc = _get_nc(t, fast_thr)
    in_maps = _prep_inputs(x, W1, b1, W2, b2, W3, b3)
    bkr = run_bass_kernel_spmd(nc, in_maps, list(range(NCORES)), **spmd_kwargs)
    res = bkr.results
    spk = np.empty((t, B, NO), np.float32)
    m3 = np.empty((B, NO), np.float32)
    for c in range(NCORES):
        spk[:, c * BC:(c + 1) * BC, :] = (
            res[c]["spk"].reshape(NO, t, BC).transpose(1, 2, 0)
        )
        m3[c * BC:(c + 1) * BC, :] = res[c]["m3"].T + 5.0 * b3[None, :]
    return (spk, m3), bkr


def kernel(x, W1, b1, W2, b2, W3, b3):
    out, _ = _run(x, W1, b1, W2, b2, W3, b3)
    return out
